# revision 25
# baseline (speedup 1.0000x reference)
"""Trainium2 Bass kernel for one transformer block (nn_Block_25838523252853).

Full inputs in, full output out. Sharding: the 4096 tokens (B=4 x L=1024)
are split 8 ways -- each core owns 512 tokens (half of one sequence).
Attention needs full-sequence K/V, so cores exchange their K/V halves with
their pair-neighbour via two AllGathers, split by head group so attention
on heads 0-7 starts while the second gather is in flight.

Device layout: activations are channel-major bf16 ([C_part, T_free] tiles),
weights in natural [inC, outC] layout as matmul lhsT. LayerNorm reductions
(over channels = partitions) use ones-vector matmuls interleaved with the
producing matmuls; per-token scalars are broadcast across partitions with
tiny K=1/K=2 matmuls whose lhsT rows carry gamma/beta, so the normalize
epilogue is two fused DVE ops per tile. Q/K/V/proj all run fp8 DoubleRow
off paired x16 LN1 copies. Attention is head-pair batched: score matmuls
are K=64 so the two heads of a pair run CONCURRENTLY in the PE array (row
groups 0/64 via auto tile_position); softmax exp is a Schraudolph bit-trick
on the vector engine (int16(s*128/ln2+16251) bit-viewed as bf16, ~3% err,
~4x an ACT Exp and leaves the scalar engine free for the 8/den Reciprocal);
row sums come free from a ones-column appended to V; softmax skips the max
subtraction (scores bounded ~|9| for this problem). fc2 runs k-outer with
w2 streamed through 4 rotating SBUF tiles (never fully resident).
"""

import numpy as np
import ml_dtypes

import concourse.bass as bass
import concourse.tile as tile
from concourse import bacc, mybir
from concourse.bass_utils import run_bass_kernel_spmd

F32 = mybir.dt.float32
BF16 = mybir.dt.bfloat16
FP16 = mybir.dt.float16
I16 = mybir.dt.int16

# Schraudolph bit-trick exp: bf16(int16(x * 128/ln2 + 16251)) ~= e^x (+-3.5%)
EXP_A = 128.0 / float(np.log(2.0))
EXP_B = 16251.0
N_BRIDGE = 24    # PE heat matmuls bridging the AG-A wait

DIM = 1024
HEADS = 16
HD = 64
HIDDEN = 4096
EPS = 1e-5
SCALE = HD ** -0.5
B, L = 4, 1024
T = 512          # tokens owned per core
P = 128
NC = 8

_BUILT = None

AF = mybir.ActivationFunctionType
ALU = mybir.AluOpType


def build():
    """Build + bacc-compile the SPMD program. Cached per process."""
    global _BUILT
    if _BUILT is not None:
        return _BUILT

    nc = bacc.Bacc("TRN2", target_bir_lowering=False, debug=False, num_devices=NC)

    d_xs = nc.dram_tensor("xs", [2 * DIM, T], BF16, kind="ExternalInput").ap()
    d_wsk = nc.dram_tensor("wsk", [2 * DIM, DIM], BF16, kind="ExternalInput").ap()
    FP8 = mybir.dt.float8e4
    # Q/K weights pre-paired fp8 (q x256 incl. softmax scale, k x32)
    d_wq = nc.dram_tensor("wq", [4 * P, 2 * DIM], FP8, kind="ExternalInput").ap()
    d_wk = nc.dram_tensor("wk", [4 * P, 2 * DIM], FP8, kind="ExternalInput").ap()
    # V/proj weights pre-paired fp8 (x32): [kpair*128, 2*1024]
    d_wv = nc.dram_tensor("wv", [4 * P, 2 * DIM], FP8, kind="ExternalInput").ap()
    d_wp = nc.dram_tensor("wp", [4 * P, 2 * DIM], FP8, kind="ExternalInput").ap()
    d_w1 = nc.dram_tensor("w1", [DIM, HIDDEN], BF16, kind="ExternalInput").ap()
    d_w2 = nc.dram_tensor("w2", [HIDDEN, DIM], BF16, kind="ExternalInput").ap()
    # proj bias row (x256) for the fp8 proj accumulation
    d_pbr = nc.dram_tensor("pbr", [1, DIM], BF16, kind="ExternalInput").ap()
    # per-channel columns: [128, 56]: skb 0:8, pb 8:16, b2 16:24, b1 24:56
    d_cols = nc.dram_tensor("cols", [P, 56], F32, kind="ExternalInput").ap()
    # gamma cols for STT epilogues: [128, 24]: g1 0:8, g2 8:16, g3 16:24
    d_gcol = nc.dram_tensor("gcol", [P, 24], F32, kind="ExternalInput").ap()
    # bcast lhsT rows, bf16: [2, 3*DIM]: cols i*DIM+c: row0 = -g_i, row1 = beta_i
    d_gb = nc.dram_tensor("gb", [2, 3 * DIM], BF16, kind="ExternalInput").ap()
    # collective bounce buffers: ccA = K chans 0:512 + V chans 0:512 (heads 0-7)
    ccW_in = nc.dram_tensor("ccW_in", [1, 64], BF16).ap()
    ccW_out = nc.dram_tensor("ccW_out", [2, 64], BF16).ap()
    ccA_in = nc.dram_tensor("ccA_in", [DIM, T], BF16).ap()
    ccA_out = nc.dram_tensor("ccA_out", [2 * DIM, T], BF16).ap()
    ccB_in = nc.dram_tensor("ccB_in", [DIM, T], BF16).ap()
    ccB_out = nc.dram_tensor("ccB_out", [2 * DIM, T], BF16).ap()
    d_out = nc.dram_tensor("out", [DIM, T], F32, kind="ExternalOutput").ap()
    DEBUG = bool(int(__import__("os").environ.get("KDBG", "0")))
    if DEBUG:
        d_dbg_q = nc.dram_tensor("dbg_q", [P, T], BF16, kind="ExternalOutput").ap()
        d_dbg_k = nc.dram_tensor("dbg_k", [P, T], BF16, kind="ExternalOutput").ap()
        d_dbg_e = nc.dram_tensor("dbg_e", [P, 2 * T], BF16, kind="ExternalOutput").ap()
        d_dbg_r = nc.dram_tensor("dbg_r", [1, T], BF16, kind="ExternalOutput").ap()
        d_dbg_u = nc.dram_tensor("dbg_u", [HD, T], BF16, kind="ExternalOutput").ap()
        d_dbg_o = nc.dram_tensor("dbg_o", [P, 2 * T], mybir.dt.float8e4,
                                 kind="ExternalOutput").ap()

    GROUPS = [[0, 1], [2, 3], [4, 5], [6, 7]]
    EXPW = 2 * T

    with tile.TileContext(nc, pool_alloc_mode="queue") as tc:
        with tc.tile_pool(name="glob", bufs=1) as gpool, \
             tc.tile_pool(name="tmp", bufs=2) as tpool, \
             tc.tile_pool(name="ps", bufs=3, space="PSUM") as ppool:

            # warm-up AllGather: pays the one-time ~10us CC mesh startup
            # during phase A instead of on the critical first real gather
            nc.gpsimd.collective_compute(
                "AllGather", ALU.bypass,
                replica_groups=[[0, 1], [2, 3], [4, 5], [6, 7]],
                ins=[ccW_in[:]], outs=[ccW_out[:]],
            )
            cols = gpool.tile([P, 56], F32, tag="cols", name="cols")
            gcol = gpool.tile([P, 24], F32, tag="gcol", name="gcol")
            gb = gpool.tile([2, 3 * DIM], BF16, tag="gb", name="gb")
            ones_b = gpool.tile([P, 1], BF16, tag="ones_b", name="ones_b")
            nc.vector.memset(ones_b, 1.0)
            ones_h = gpool.tile([P, 1], FP16, tag="ones_h", name="ones_h")
            nc.vector.memset(ones_h, 1.0)
            ones_row = gpool.tile([1, P], BF16, tag="ones_row", name="ones_row")
            nc.vector.memset(ones_row, 1.0)
            eights_row = gpool.tile([1, P], BF16, tag="e8row", name="e8row")
            nc.vector.memset(eights_row, 8.0)
            ones_t = gpool.tile([1, T], BF16, tag="ones_t", name="ones_t")
            nc.vector.memset(ones_t, 1.0)
            zrow = gpool.tile([1, P], BF16, tag="zrow", name="zrow")
            nc.vector.memset(zrow, 0.0)
            pbr = gpool.tile([1, DIM], BF16, tag="pbr", name="pbr")
            eps_t = gpool.tile([1, 1], F32, tag="eps_t", name="eps_t")
            nc.vector.memset(eps_t, EPS)
            # per-LN broadcast rhs [2, T]: row0 = mean*rstd (written), row1 = 1
            brh = [gpool.tile([2, T], BF16, tag=f"brh{i}", name=f"brh{i}")
                   for i in range(3)]
            for i in range(3):
                # whole-tile memset (1-partition access at base 1 is illegal);
                # row 0 is overwritten with mean*rstd by each LN chain
                nc.vector.memset(brh[i], 1.0)
            # long-lived activation tiles (hT aliases x1n/oT slots later)
            x1n = [gpool.tile([P, T], BF16, tag=f"x1n_{k}", name=f"x1n_{k}")
                   for k in range(8)]
            x2n = [gpool.tile([P, T], BF16, tag=f"x2n{m}", name=f"x2n{m}")
                   for m in range(8)]
            oTf = [gpool.tile([P, 2 * T], FP8, tag=f"oTf{i}", name=f"oTf{i}")
                   for i in range(4)]

            wp = [gpool.tile([P, 2 * DIM], FP8, tag=f"wp{k}", name=f"wp{k}")
                  for k in range(4)]
            x1f = [gpool.tile([P, 2 * T], FP8, tag=f"x1f{i}", name=f"x1f{i}")
                   for i in range(4)]

            C_SKB, C_PRB, C_F2B, C_F1B = 0, 8, 16, 24

            def emit_stats(stats, raw_m, sq_m, m, nk=8):
                nc.tensor.matmul(stats[0:1, :], lhsT=ones_b, rhs=raw_m,
                                 start=(m == 0), stop=(m == nk - 1))
                nc.tensor.matmul(stats[32:33, :], lhsT=ones_h, rhs=sq_m,
                                 start=(m == 0), stop=(m == nk - 1))

            def emit_ln_chain(stats, ln_i, n_feat, warm_rhs=None, warm_n=12):
                """LN stats -> (a_sb [128,T] bf16 = rstd bcast, make_b(m) fn)."""
                inv_n = 1.0 / n_feat
                if warm_rhs is not None:
                    # keep the PE HAM clock warm through the serial chain
                    hch = ppool.tile([P, T], F32, tag="mm", name="lnheat")
                    for i in range(warm_n):
                        nc.tensor.matmul(hch[0:1, :], lhsT=ones_b, rhs=warm_rhs,
                                         start=(i == 0), stop=(i == warm_n - 1))
                msq = tpool.tile([1, T], F32, tag="ln_msq", name="ln_msq", bufs=1)
                nc.scalar.activation(msq, stats[0:1, :], AF.Square, scale=inv_n)
                var = tpool.tile([1, T], F32, tag="ln_var", name="ln_var", bufs=1)
                nc.vector.scalar_tensor_tensor(var, stats[32:33, :], inv_n, msq,
                                               ALU.mult, ALU.subtract)
                lnv = tpool.tile([1, T], F32, tag="ln_lnv", name="ln_lnv", bufs=1)
                nc.scalar.activation(lnv, var, AF.Ln, bias=eps_t)
                rstd = tpool.tile([1, T], F32, tag="ln_rstd", name="ln_rstd",
                                  bufs=1)
                nc.scalar.activation(rstd, lnv, AF.Exp, scale=-0.5)
                rstd_bf = tpool.tile([1, T], BF16, tag="ln_rstdb", name="ln_rstdb",
                                     bufs=1)
                nc.vector.tensor_copy(out=rstd_bf, in_=rstd)
                # brh row0 = mean * rstd  (bf16)
                nc.vector.scalar_tensor_tensor(brh[ln_i][0:1, :], stats[0:1, :],
                                               inv_n, rstd, ALU.mult, ALU.mult)
                a_ps = ppool.tile([P, T], F32, tag="mm", name="mm")
                nc.tensor.matmul(a_ps, lhsT=ones_row, rhs=rstd_bf, start=True,
                                 stop=True)
                a_sb = tpool.tile([P, T], BF16, tag="ln_asb", name="ln_asb", bufs=1)
                nc.vector.tensor_copy(out=a_sb, in_=a_ps)

                def make_b(m):
                    b_ps = ppool.tile([P, T], F32, tag="mm", name="mm")
                    if warm_rhs is None:
                        nc.tensor.matmul(b_ps, lhsT=gb[:, ln_i * DIM + m * P:
                                                       ln_i * DIM + (m + 1) * P],
                                         rhs=brh[ln_i], start=True, stop=True)
                    else:
                        # zero-weight fills keep the clock up through the
                        # DVE-paced finals without touching the data
                        nc.tensor.matmul(b_ps, lhsT=gb[:, ln_i * DIM + m * P:
                                                       ln_i * DIM + (m + 1) * P],
                                         rhs=brh[ln_i], start=True, stop=False)
                        for fi in range(2):
                            nc.tensor.matmul(b_ps, lhsT=zrow,
                                             rhs=warm_rhs[0:1, :],
                                             start=False, stop=(fi == 1))
                    return b_ps
                return a_sb, make_b

            def emit_ln_final(raw_m, a_sb, b_ps, ln_i, m, out_tile):
                """out = g*(raw*rstd) + (-g*mr + beta), two DVE ops."""
                t1 = tpool.tile([P, T], BF16, tag="ln_t1", name="ln_t1", bufs=2)
                nc.vector.tensor_tensor(t1, raw_m, a_sb, ALU.mult)
                nc.vector.scalar_tensor_tensor(
                    out_tile, t1, gcol[:, 8 * ln_i + m:8 * ln_i + m + 1], b_ps,
                    ALU.mult, ALU.add)

            # =========== Phase A: skip-concat linear + LN1 stats ===========
            wqkvp = tc.alloc_tile_pool(name="wqkv", bufs=1)
            with tc.tile_pool(name="pha", bufs=1) as apool:
                wsk = [apool.tile([P, DIM], BF16, tag=f"wsk{k}", name=f"wsk{k}")
                       for k in range(16)]
                xs = [apool.tile([P, T], BF16, tag=f"xsh{k}", name=f"xsh{k}")
                      for k in range(16)]
                for k in range(16):
                    eng = nc.sync if k % 2 == 0 else nc.gpsimd
                    eng.dma_start(out=wsk[k], in_=d_wsk[k * P:(k + 1) * P, :])
                    eng.dma_start(out=xs[k], in_=d_xs[k * P:(k + 1) * P, :])
                    if k == 0:
                        nc.gpsimd.dma_start(out=cols, in_=d_cols)
                        nc.gpsimd.dma_start(out=gcol, in_=d_gcol)
                        nc.gpsimd.dma_start(out=gb, in_=d_gb)
                        nc.gpsimd.dma_start(out=pbr, in_=d_pbr)
                # prefetch q/k/v weights (all fp8 paired) behind phase-A tiles
                wq = [wqkvp.tile([P, 2 * DIM], FP8, tag=f"wq{k}", name=f"wq{k}")
                      for k in range(4)]
                wk = [wqkvp.tile([P, 2 * DIM], FP8, tag=f"wk{k}", name=f"wk{k}")
                      for k in range(4)]
                wv = [wqkvp.tile([P, 2 * DIM], FP8, tag=f"wv{k}", name=f"wv{k}")
                      for k in range(4)]
                for k in range(4):
                    nc.sync.dma_start(out=wk[k], in_=d_wk[k * P:(k + 1) * P, :])
                    nc.gpsimd.dma_start(out=wv[k], in_=d_wv[k * P:(k + 1) * P, :])
                    nc.sync.dma_start(out=wq[k], in_=d_wq[k * P:(k + 1) * P, :])

                raw = [apool.tile([P, T], BF16, tag=f"raw{m}", name=f"raw{m}")
                       for m in range(8)]
                sq = [apool.tile([P, T], FP16, tag=f"sq{m}", name=f"sq{m}")
                      for m in range(8)]
                # hybrid k-outer: ride the input DMA stream with all 8 psum
                # slots, then finish per-m so the drains stagger
                aslots = [ppool.tile([P, EXPW], F32, tag="big", bufs=2,
                                     name="abig") for _ in range(2)]
                aslots += [ppool.tile([P, T], F32, tag="mm", name="amm")
                           for _ in range(3)]
                aslots += [ppool.tile([P, T], F32, tag="st", name="ast", bufs=1)]

                def aslot(m):
                    if m < 4:
                        return aslots[m // 2][:, (m % 2) * T:(m % 2 + 1) * T]
                    return aslots[2 + (m - 4)]

                for k in range(12):
                    for m in range(8):
                        nc.tensor.matmul(
                            aslot(m), lhsT=wsk[k][:, m * P:(m + 1) * P],
                            rhs=xs[k], start=(k == 0), stop=False)
                stats1 = None
                for m in range(8):
                    for k in range(12, 16):
                        nc.tensor.matmul(
                            aslot(m), lhsT=wsk[k][:, m * P:(m + 1) * P],
                            rhs=xs[k], start=False, stop=(k == 15))
                    # raw = psum + skip_b (per-partition col), bf16
                    nc.vector.tensor_scalar(raw[m], aslot(m),
                                            cols[:, C_SKB + m:C_SKB + m + 1], None,
                                            ALU.add)
                    nc.scalar.activation(sq[m], raw[m], AF.Square)
                    if m == 7:
                        stats1 = ppool.tile([P, T], F32, tag="st", name="st1",
                                            bufs=1)
                    if stats1 is not None:
                        if m == 7:
                            for m2 in range(8):
                                emit_stats(stats1, raw[m2], sq[m2], m2)

                # ---- LN1 chain + finals (finals pipeline into K below) ----
                a1, make_b1 = emit_ln_chain(stats1, 0, DIM, warm_rhs=raw[0], warm_n=16)
                for m in range(8):
                    emit_ln_final(raw[m], a1, make_b1(m), 0, m, x1n[m])
                    # x16 fp8 copy of x1n pairs for QKV/proj DoubleRow
                    nc.vector.tensor_scalar(
                        x1f[m // 2][:, (m % 2) * T:(m % 2 + 1) * T],
                        x1n[m], 16.0, None, ALU.mult)

            # =========== Phase B: K, V halves + AllGathers, Q ===========
            with tc.tile_pool(name="phb", bufs=1) as bpool:
                cc_in = [ccA_in, ccB_in]
                cc_out = [ccA_out, ccB_out]
                kloc = [bpool.tile([P, T], BF16, tag=f"kl{m}", name=f"kl{m}")
                        for m in range(8)]
                vloc = [bpool.tile([P, T], BF16, tag=f"vl{i}", name=f"vl{i}")
                        for i in range(8)]
                qT = [bpool.tile([P, T], BF16, tag=f"qT{m}", name=f"qT{m}")
                      for m in range(8)]

                def emit_k(m, half):
                    # K tile m, fp8 DoubleRow over paired x1 channels
                    ps = ppool.tile([P, T], F32, tag="mm", name="mm")
                    for kp in range(4):
                        nc.tensor.matmul(
                            ps,
                            lhsT=wk[kp].rearrange("p (j c) -> p j c", j=2)
                            [:, :, m * P:(m + 1) * P],
                            rhs=x1f[kp].rearrange("p (j t) -> p j t", j=2),
                            start=(kp == 0), stop=(kp == 3),
                            perf_mode=mybir.MatmulPerfMode.DoubleRow)
                    nc.vector.tensor_scalar(kloc[m], ps, 1.0 / 512, None,
                                            ALU.mult)
                    nc.sync.dma_start(
                        out=cc_in[half][(m - 4 * half) * P:
                                        (m - 4 * half + 1) * P, :],
                        in_=kloc[m])

                def emit_v(kt, half):
                    ps = ppool.tile([P, T], F32, tag="mm", name="mm")
                    for kp in range(4):
                        nc.tensor.matmul(
                            ps,
                            lhsT=x1f[kp].rearrange("p (j t) -> p j t", j=2)
                            [:, :, kt * P:(kt + 1) * P],
                            rhs=wv[kp].rearrange("p (j c) -> p j c", j=2)
                            [:, :, half * T:(half + 1) * T],
                            start=(kp == 0), stop=(kp == 3),
                            perf_mode=mybir.MatmulPerfMode.DoubleRow)
                    nc.vector.tensor_scalar(vloc[half * 4 + kt], ps, 1.0 / 512,
                                            None, ALU.mult)
                    nc.sync.dma_start(
                        out=cc_in[half][T + kt * P:T + (kt + 1) * P, :],
                        in_=vloc[half * 4 + kt])

                # K half0 + V half0 feed AG-A as early as possible
                for m in range(4):
                    emit_k(m, 0)
                for kt in range(4):
                    emit_v(kt, 0)
                nc.gpsimd.collective_compute(
                    "AllGather", ALU.bypass, replica_groups=GROUPS,
                    ins=[cc_in[0][:]], outs=[cc_out[0][:]],
                )
                # Q for own tokens (overlaps AG-A), fp8 DoubleRow
                for m in range(8):
                    ps = ppool.tile([P, T], F32, tag="mm", name="mm")
                    for kp in range(4):
                        nc.tensor.matmul(
                            ps,
                            lhsT=wq[kp].rearrange("p (j c) -> p j c", j=2)
                            [:, :, m * P:(m + 1) * P],
                            rhs=x1f[kp].rearrange("p (j t) -> p j t", j=2),
                            start=(kp == 0), stop=(kp == 3),
                            perf_mode=mybir.MatmulPerfMode.DoubleRow)
                    nc.vector.tensor_scalar(qT[m], ps, 1.0 / 4096, None,
                                            ALU.mult)
                # K half1 + V half1 + AG-B, still ahead of attention
                for m in range(4, 8):
                    emit_k(m, 1)
                for kt in range(4):
                    emit_v(kt, 1)
                nc.gpsimd.collective_compute(
                    "AllGather", ALU.bypass, replica_groups=GROUPS,
                    ins=[cc_in[1][:]], outs=[cc_out[1][:]],
                )

                # reload gathered K/V (uniform across cores; k-token order is
                # attention-invariant). kT[m][b]: chans m*128.., token block b.
                kT = [[bpool.tile([P, T], BF16, tag=f"kT_{m}_{b}",
                                  name=f"kT_{m}_{b}")
                       for b in range(2)] for m in range(8)]
                # per-half V tiles: heads 0-7 tiles depend only on AG-A so
                # attention on heads 0-7 never waits for AG-B
                v_hf = [[bpool.tile([P, 8 * (HD + 1)], BF16, tag=f"v{half}_{kt}",
                                    name=f"v{half}_{kt}")
                         for kt in range(8)] for half in range(2)]
                for half in range(2):
                    for kt in range(8):
                        v3 = v_hf[half][kt].rearrange("p (h c) -> p h c", c=HD + 1)
                        nc.vector.memset(v3[:, :, HD:HD + 1], 1.0)

                def emit_reload(half):
                    for b in range(2):
                        for mi in range(4):
                            m = half * 4 + mi
                            nc.sync.dma_start(
                                out=kT[m][b],
                                in_=cc_out[half][b * DIM + mi * P:
                                                 b * DIM + (mi + 1) * P, :])
                        for ktl in range(4):
                            kt = b * 4 + ktl
                            v3 = v_hf[half][kt].rearrange("p (h c) -> p h c",
                                                          c=HD + 1)
                            nc.sync.dma_start(
                                out=v3[:, :, 0:HD],
                                in_=cc_out[half][b * DIM + T + ktl * P:
                                                 b * DIM + T + (ktl + 1) * P, :]
                                .rearrange("p (h c) -> p h c", c=HD))
                emit_reload(0)

                for k in range(4):
                    nc.gpsimd.dma_start(out=wp[k], in_=d_wp[k * P:(k + 1) * P, :])

                emit_reload(1)

                # =========== Phase C: attention, head-pair batched ===========
                # per pair m2 (heads 2m2, 2m2+1): 8 score slots of 2 concurrent
                # K=64 matmuls (row groups 0/64), Schraudolph exp on DVE into
                # int16-as-bf16, bf16 attnV with sums column, DVE reciprocal
                # straight off PSUM, eights-row broadcast matmul, oTf from
                # PSUM x PSUM.
                with tc.tile_pool(name="exps", bufs=8) as xpool:
                    # heat bridge while AG-A lands (PE would idle > HAM window)
                    heat = ppool.tile([P, T], F32, tag="st", name="heat", bufs=1)
                    for i in range(N_BRIDGE):
                        nc.tensor.matmul(heat, lhsT=qT[0][:, 0:P], rhs=qT[0],
                                         start=(i == 0), stop=(i == N_BRIDGE - 1))

                    pend = None   # (m2, poA, poB, rhbA, rhbB) awaiting bc/oTf

                    def emit_scores(m2):
                        es = []
                        for j in range(8):
                            th, col = j // 4, (j % 4) * P
                            big = ppool.tile([P, EXPW], F32, tag="big", bufs=2,
                                             name="sc")
                            nc.tensor.matmul(
                                big[:, 0:T],
                                lhsT=kT[m2][th][0:HD, col:col + P],
                                rhs=qT[m2][0:HD, :], start=True, stop=True)
                            nc.tensor.matmul(
                                big[:, T:2 * T],
                                lhsT=kT[m2][th][HD:P, col:col + P],
                                rhs=qT[m2][HD:P, :], start=True, stop=True)
                            e = xpool.tile([P, EXPW], BF16, tag="exp", name="exp")
                            nc.vector.tensor_scalar(e.bitcast(I16), big, EXP_A,
                                                    EXP_B, ALU.mult, ALU.add)
                            es.append(e)
                        return es

                    def emit_attnv(m2, es):
                        half = m2 // 4
                        pos = []
                        for hh2 in range(2):
                            hh = (m2 % 4) * 2 + hh2
                            po = ppool.tile([P, T], F32, tag="mm", name="mm")
                            for j in range(8):
                                nc.tensor.matmul(
                                    po[0:HD + 1, :],
                                    lhsT=v_hf[half][j]
                                    [:, hh * (HD + 1):(hh + 1) * (HD + 1)],
                                    rhs=es[j][:, hh2 * T:(hh2 + 1) * T],
                                    start=(j == 0), stop=(j == 7))
                            # den must reach a base-0 tile before the custom
                            # recip op (it mishandles base_partition != 0);
                            # stage it + the o_unnorm drain on the idle ACT
                            den = tpool.tile([1, T], F32, tag=f"den{hh2}",
                                             name="den", bufs=2)
                            nc.scalar.activation(den, po[HD:HD + 1, :], AF.Copy)
                            rh = tpool.tile([1, T], F32, tag=f"rh{hh2}",
                                            name="rh", bufs=2)
                            nc.vector.reciprocal_approx_fast(out=rh, in_=den)
                            rhb = tpool.tile([1, T], BF16, tag=f"rhb{hh2}",
                                             name="rhb", bufs=2)
                            nc.scalar.activation(rhb, rh, AF.Copy)
                            oU = tpool.tile([HD, T], BF16, tag="oub",
                                            name="oub", bufs=4)
                            nc.scalar.activation(oU, po[0:HD, :], AF.Copy)
                            pos.append((oU, rhb))
                        return (m2, pos[0][0], pos[1][0], pos[0][1], pos[1][1])

                    def finish_pair(m2, oUA, oUB, rhbA, rhbB):
                        # bc = 8/den broadcast; oTf = (o_unnorm/den)*8 in fp8
                        bcA = ppool.tile([P, T], F32, tag="st", name="bcA",
                                         bufs=1)
                        nc.tensor.matmul(bcA[0:HD, :], lhsT=eights_row[:, 0:HD],
                                         rhs=rhbA, start=True, stop=True)
                        bcB = ppool.tile([P, T], F32, tag="mm", name="bcB")
                        nc.tensor.matmul(bcB[0:HD, :], lhsT=eights_row[:, 0:HD],
                                         rhs=rhbB, start=True, stop=True)
                        sl = slice((m2 % 2) * T, (m2 % 2 + 1) * T)
                        nc.vector.tensor_tensor(
                            oTf[m2 // 2][0:HD, sl], oUA, bcA[0:HD, :],
                            ALU.mult)
                        nc.vector.tensor_tensor(
                            oTf[m2 // 2][HD:P, sl], oUB, bcB[0:HD, :],
                            ALU.mult)

                    for m2 in range(8):
                        es = emit_scores(m2)
                        if DEBUG and m2 == 0:
                            nc.gpsimd.dma_start(out=d_dbg_q, in_=qT[0])
                            nc.gpsimd.dma_start(out=d_dbg_k, in_=kT[0][0])
                            nc.gpsimd.dma_start(out=d_dbg_e, in_=es[0])
                        if pend is not None:
                            finish_pair(*pend)
                        pend = emit_attnv(m2, es)
                        if DEBUG and m2 == 0:
                            nc.gpsimd.dma_start(out=d_dbg_r, in_=pend[3])
                            nc.gpsimd.dma_start(out=d_dbg_u, in_=pend[1])
                    finish_pair(*pend)
                    if DEBUG:
                        nc.gpsimd.dma_start(out=d_dbg_o, in_=oTf[0])

                    # proj k-outer over 5 psum slots starts while the last
                    # head's epilogue drains
                    pj = [ppool.tile([P, EXPW], F32, tag="big", bufs=2,
                                     name="pjbig") for _ in range(2)]
                    pj += [ppool.tile([P, T], F32, tag="st", name="pjst", bufs=1)]

                    def pjslot(m):
                        if m < 4:
                            return pj[m // 2][:, (m % 2) * T:(m % 2 + 1) * T]
                        return pj[2]

                    for kp in range(4):
                        for m in range(5):
                            nc.tensor.matmul(
                                pjslot(m),
                                lhsT=wp[kp].rearrange("p (j c) -> p j c", j=2)
                                [:, :, m * P:(m + 1) * P],
                                rhs=oTf[kp].rearrange("p (j t) -> p j t", j=2),
                                start=(kp == 0), stop=False,
                                perf_mode=mybir.MatmulPerfMode.DoubleRow)
                    for m in range(5):
                        # + 256*proj_b via bf16 K=1 row (closes the group)
                        nc.tensor.matmul(pjslot(m), lhsT=pbr[:, m * P:(m + 1) * P],
                                         rhs=ones_t, start=False, stop=True)
            wqkvp.release()

            # =========== Phase D: proj m5..7 + residual + LN2 ===========
            w1pool = tc.alloc_tile_pool(name="w1p", bufs=1)
            w1 = [w1pool.tile([P, HIDDEN], BF16, tag=f"w1{k}", name=f"w1{k}")
                  for k in range(8)]
            for k in range(8):
                eng = nc.sync if k % 2 == 0 else nc.gpsimd
                eng.dma_start(out=w1[k], in_=d_w1[k * P:(k + 1) * P, :])
            with tc.tile_pool(name="phd", bufs=1) as dpool:
                x2r = [dpool.tile([P, T], BF16, tag=f"x2r{m}", name=f"x2r{m}")
                       for m in range(8)]
                x2sq = [dpool.tile([P, T], FP16, tag=f"x2sq{m}", name=f"x2sq{m}")
                        for m in range(8)]
                pj5 = [None] * 3
                for mi in range(3):
                    m = 5 + mi
                    ps = ppool.tile([P, T], F32, tag="mm", name="mm")
                    for kp in range(4):
                        nc.tensor.matmul(
                            ps,
                            lhsT=wp[kp].rearrange("p (j c) -> p j c", j=2)
                            [:, :, m * P:(m + 1) * P],
                            rhs=oTf[kp].rearrange("p (j t) -> p j t", j=2),
                            start=(kp == 0), stop=False,
                            perf_mode=mybir.MatmulPerfMode.DoubleRow)
                    nc.tensor.matmul(ps, lhsT=pbr[:, m * P:(m + 1) * P],
                                     rhs=ones_t, start=False, stop=True)
                    pj5[mi] = ps
                stats2 = None
                for m in range(8):
                    ps = pjslot(m) if m < 5 else pj5[m - 5]
                    # x2r = psum/256 + x1n (psum carries 256*proj_b)
                    nc.vector.scalar_tensor_tensor(
                        x2r[m], ps, 1.0 / 256, x1n[m], ALU.mult, ALU.add)
                    nc.scalar.activation(x2sq[m], x2r[m], AF.Square)
                    if m == 0:
                        stats2 = ppool.tile([P, T], F32, tag="st", name="st2",
                                            bufs=1)
                    emit_stats(stats2, x2r[m], x2sq[m], m)

                a2, make_b2 = emit_ln_chain(stats2, 1, DIM, warm_rhs=x2r[0])
                for m in range(8):
                    emit_ln_final(x2r[m], a2, make_b2(m), 1, m, x2n[m])

            # =========== Phase E: MLP + LN3 ===========
            with tc.tile_pool(name="phe", bufs=1) as epool:
                hT = [epool.tile([P, T], BF16, tag=f"hT{i}", name=f"hT{i}")
                      for i in range(32)]
                pf = None
                for mm in range(32):
                    if mm < 4:
                        # k-outer for the first 4 out-tiles (pipelines with LN2)
                        if mm == 0:
                            pf = [ppool.tile([P, EXPW], F32, tag="big", bufs=2,
                                             name="fbig") for _ in range(2)]
                            for k in range(8):
                                for j in range(4):
                                    nc.tensor.matmul(
                                        pf[j // 2][:, (j % 2) * T:(j % 2 + 1) * T],
                                        lhsT=w1[k][:, j * P:(j + 1) * P],
                                        rhs=x2n[k], start=(k == 0), stop=(k == 7))
                        ps = pf[mm // 2][:, (mm % 2) * T:(mm % 2 + 1) * T]
                    else:
                        ps = ppool.tile([P, T], F32, tag="mm", name="mm")
                        for k in range(8):
                            nc.tensor.matmul(ps, lhsT=w1[k][:, mm * P:(mm + 1) * P],
                                             rhs=x2n[k], start=(k == 0),
                                             stop=(k == 7))
                    nc.scalar.activation(hT[mm], ps, AF.Gelu,
                                         bias=cols[:, C_F1B + mm:C_F1B + mm + 1])

                # ---- fc2: two k-outer passes of 4 out-tiles, w2 streamed
                # through 4 rotating SBUF tiles (re-read on pass 2); pass-1
                # drains overlap pass-2 matmuls ----
                x3sq = [epool.tile([P, T], FP16, tag=f"x3sq{m}", name=f"x3sq{m}")
                        for m in range(8)]
                x3r = [epool.tile([P, T], BF16, tag=f"x3r{m}", name=f"x3r{m}")
                       for m in range(8)]
                f2ps = {}
                for p2 in range(2):
                    if p2 == 0:
                        slots = [ppool.tile([P, EXPW], F32, tag="big", bufs=2,
                                            name="f2big") for _ in range(2)]
                        sl = lambda m: slots[m // 2][:, (m % 2) * T:
                                                     (m % 2 + 1) * T]
                    else:
                        slots = [ppool.tile([P, T], F32, tag="mm", name="f2mm")
                                 for _ in range(3)]
                        slots += [ppool.tile([P, T], F32, tag="st", name="f2st",
                                             bufs=1)]
                        sl = lambda m: slots[m - 4]
                    for k in range(32):
                        w2t = epool.tile([P, T], BF16, tag=f"w2r{k % 4}",
                                         name=f"w2_{p2}_{k}")
                        eng = nc.sync if k % 2 == 0 else nc.gpsimd
                        eng.dma_start(out=w2t,
                                      in_=d_w2[k * P:(k + 1) * P,
                                               p2 * T:(p2 + 1) * T])
                        for mi in range(4):
                            m = p2 * 4 + mi
                            nc.tensor.matmul(sl(m),
                                             lhsT=w2t[:, mi * P:(mi + 1) * P],
                                             rhs=hT[k], start=(k == 0),
                                             stop=(k == 31))
                    if p2 == 0:
                        # stats live in a freed pass-1 "big" bank so the
                        # stats matmuls interleave with pass-2 matmuls
                        stats3 = ppool.tile([P, EXPW], F32, tag="big", bufs=2,
                                            name="st3")[:, 0:T]
                    for mi in range(4):
                        m = p2 * 4 + mi
                        f2ps[m] = sl(m)
                        nc.vector.scalar_tensor_tensor(
                            x3r[m], f2ps[m], cols[:, C_F2B + m:C_F2B + m + 1],
                            x2n[m], ALU.add, ALU.add)
                        nc.scalar.activation(x3sq[m], x3r[m], AF.Square)
                        emit_stats(stats3, x3r[m], x3sq[m], m)

                a3, make_b3 = emit_ln_chain(stats3, 2, DIM, warm_rhs=x3r[0], warm_n=24)
                for m in range(8):
                    xo = tpool.tile([P, T], F32, tag="xo", name="xo", bufs=2)
                    emit_ln_final(x3r[m], a3, make_b3(m), 2, m, xo)
                    eng = nc.sync if m % 2 == 0 else nc.gpsimd
                    eng.dma_start(out=d_out[m * P:(m + 1) * P, :], in_=xo)
            w1pool.release()

    # Steer the act-table selector: keep dict ORDER (act_func_set_id is the
    # positional index into act_info.json) but hide Exp/Ln from the small
    # tables so both resolve to the combined natural_log_exp_and_others set
    # and the attention/LN loop stops thrashing table loads.
    import concourse.hw_specs as hw_specs
    tabs = dict(hw_specs.get_activation_tables("gen3"))
    steered = {}
    for name, fns in tabs.items():
        fns = set(fns)
        if name != "natural_log_exp_and_others":
            fns.discard(AF.Exp)
            fns.discard(AF.Ln)
        steered[name] = fns
    import functools
    _orig = hw_specs.get_activation_tables
    patched = functools.lru_cache(None)(
        lambda arch: steered if arch == "gen3" else _orig(arch))
    hw_specs.get_activation_tables = patched
    import concourse.bacc as bacc_mod
    bacc_mod.get_activation_tables = patched

    nc.compile()
    _BUILT = nc
    return nc


def _pack_col(vec, ncols):
    """[N] per-channel vector -> [128, N//128] tile layout (channel c -> [c%128, c//128])."""
    return np.ascontiguousarray(vec.reshape(ncols, P).T.astype(np.float32))


def _prep_in_maps(inputs):
    bf = ml_dtypes.bfloat16
    x = np.asarray(inputs["x"], np.float32)
    skip = np.asarray(inputs["skip"], np.float32)
    xs = np.concatenate([x, skip], axis=2)          # [4, 1024, 2048]

    wsk = np.asarray(inputs["skip_w"], np.float32).astype(bf)
    qkv = np.asarray(inputs["qkv_w"], np.float32)
    f8 = ml_dtypes.float8_e4m3

    def pack8(w, s):
        return np.ascontiguousarray(
            (w * s).reshape(4, 2, P, DIM)
            .transpose(0, 2, 1, 3).reshape(4 * P, 2 * DIM)).astype(f8)

    wq = pack8(qkv[:, :DIM] * SCALE, 256.0)
    wk = pack8(qkv[:, DIM:2 * DIM], 32.0)
    wv = pack8(qkv[:, 2 * DIM:], 32.0)
    wp = np.ascontiguousarray(
        (np.asarray(inputs["proj_w"], np.float32) * 32.0).reshape(4, 2, P, DIM)
        .transpose(0, 2, 1, 3).reshape(4 * P, 2 * DIM)).astype(f8)
    w1 = np.asarray(inputs["fc1_w"], np.float32).astype(bf)
    w2 = np.asarray(inputs["fc2_w"], np.float32).astype(bf)
    pbrow = (np.asarray(inputs["proj_b"], np.float32) * 256.0)[None, :].astype(bf)

    cols = np.zeros((P, 56), np.float32)
    cols[:, 0:8] = _pack_col(np.asarray(inputs["skip_b"], np.float32), 8)
    cols[:, 8:16] = _pack_col(np.asarray(inputs["proj_b"], np.float32), 8)
    cols[:, 16:24] = _pack_col(np.asarray(inputs["fc2_b"], np.float32), 8)
    cols[:, 24:56] = _pack_col(np.asarray(inputs["fc1_b"], np.float32), 32)

    gcolv = np.zeros((P, 24), np.float32)
    gcolv[:, 0:8] = _pack_col(np.asarray(inputs["ln1_g"], np.float32), 8)
    gcolv[:, 8:16] = _pack_col(np.asarray(inputs["ln2_g"], np.float32), 8)
    gcolv[:, 16:24] = _pack_col(np.asarray(inputs["ln3_g"], np.float32), 8)

    gbv = np.zeros((2, 3 * DIM), np.float32)
    for i, (gk, bk) in enumerate([("ln1_g", "ln1_b"), ("ln2_g", "ln2_b"),
                                  ("ln3_g", "ln3_b")]):
        gbv[0, i * DIM:(i + 1) * DIM] = -np.asarray(inputs[gk], np.float32)
        gbv[1, i * DIM:(i + 1) * DIM] = np.asarray(inputs[bk], np.float32)

    in_maps = []
    for c in range(NC):
        b, h = c // 2, c % 2
        seq = xs[b][h * T:(h + 1) * T]               # own 512 tokens
        xsT = np.ascontiguousarray(seq.T).astype(bf)  # [2048, 512]
        in_maps.append({
            "xs": xsT, "wsk": wsk, "wq": wq, "wk": wk, "wv": wv,
            "wp": wp, "w1": w1, "w2": w2, "pbr": pbrow, "cols": cols,
            "gcol": gcolv, "gb": gbv.astype(bf),
        })
    return in_maps


def run(inputs, trace=False, trace_kwargs=None):
    nc = build()
    in_maps = _prep_in_maps(inputs)
    res = run_bass_kernel_spmd(nc, in_maps, core_ids=list(range(NC)),
                               trace=trace, **(trace_kwargs or {}))
    full = np.empty((B, L, DIM), np.float32)
    for c in range(NC):
        b, h = c // 2, c % 2
        full[b, h * T:(h + 1) * T, :] = res.results[c]["out"].T
    return full, res


def kernel(**inputs):
    out, _ = run(inputs, trace=False)
    return out



# revision 29
# speedup vs baseline: 1.1747x; 1.1747x over previous
"""Trainium2 Bass kernel for one transformer block (nn_Block_25838523252853).

Full inputs in, full output out. Sharding: the 4096 tokens (B=4 x L=1024)
are split 8 ways -- each core owns 512 tokens (half of one sequence).
Attention needs full-sequence K/V, so cores exchange their K/V halves with
their pair-neighbour via two AllGathers, split by head group so attention
on heads 0-7 starts while the second gather is in flight.

Device layout: activations are channel-major bf16 ([C_part, T_free] tiles),
weights in natural [inC, outC] layout as matmul lhsT. LayerNorm reductions
(over channels = partitions) use ones-vector matmuls interleaved with the
producing matmuls; per-token scalars are broadcast across partitions with
tiny K=1/K=2 matmuls whose lhsT rows carry gamma/beta, so the normalize
epilogue is two fused DVE ops per tile. Q/K/V/proj all run fp8 DoubleRow
off paired x16 LN1 copies. Attention is head-pair batched: score matmuls
are K=64 so the two heads of a pair run CONCURRENTLY in the PE array (row
groups 0/64 via auto tile_position); softmax exp is a Schraudolph bit-trick
on the vector engine (int16(s*128/ln2+16251) bit-viewed as bf16, ~3% err,
~4x an ACT Exp and leaves the scalar engine free for the 8/den Reciprocal);
row sums come free from a ones-column appended to V; softmax skips the max
subtraction (scores bounded ~|9| for this problem). fc2 runs k-outer with
w2 streamed through 4 rotating SBUF tiles (never fully resident).
"""

import numpy as np
import ml_dtypes

import concourse.bass as bass
import concourse.tile as tile
from concourse import bacc, mybir
from concourse.bass_utils import run_bass_kernel_spmd

F32 = mybir.dt.float32
F32R = mybir.dt.float32r
BF16 = mybir.dt.bfloat16
FP16 = mybir.dt.float16
I16 = mybir.dt.int16

# Schraudolph bit-trick exp: bf16(int16(x * 128/ln2 + 16251)) ~= e^x (+-3.5%)
EXP_A = 128.0 / float(np.log(2.0))
EXP_B = 16251.0
N_BRIDGE = 24    # PE heat matmuls bridging the AG-A wait

DIM = 1024
HEADS = 16
HD = 64
HIDDEN = 4096
EPS = 1e-5
SCALE = HD ** -0.5
B, L = 4, 1024
T = 512          # tokens owned per core
P = 128
NC = 8

_BUILT = None

AF = mybir.ActivationFunctionType
ALU = mybir.AluOpType


def build():
    """Build + bacc-compile the SPMD program. Cached per process."""
    global _BUILT
    if _BUILT is not None:
        return _BUILT

    nc = bacc.Bacc("TRN2", target_bir_lowering=False, debug=False, num_devices=NC)

    d_xs = nc.dram_tensor("xs", [2 * DIM, T], BF16, kind="ExternalInput").ap()
    d_wsk = nc.dram_tensor("wsk", [2 * DIM, DIM], BF16, kind="ExternalInput").ap()
    FP8 = mybir.dt.float8e4
    # Q/K weights pre-paired fp8 (q x256 incl. softmax scale, k x32)
    d_wq = nc.dram_tensor("wq", [4 * P, 2 * DIM], FP8, kind="ExternalInput").ap()
    d_wk = nc.dram_tensor("wk", [4 * P, 2 * DIM], FP8, kind="ExternalInput").ap()
    # V/proj weights pre-paired fp8 (x32): [kpair*128, 2*1024]
    d_wv = nc.dram_tensor("wv", [4 * P, 2 * DIM], FP8, kind="ExternalInput").ap()
    d_wp = nc.dram_tensor("wp", [4 * P, 2 * DIM], FP8, kind="ExternalInput").ap()
    d_w1 = nc.dram_tensor("w1", [DIM, HIDDEN], BF16, kind="ExternalInput").ap()
    d_w2 = nc.dram_tensor("w2", [HIDDEN, DIM], BF16, kind="ExternalInput").ap()
    # proj bias row (x256) for the fp8 proj accumulation
    d_pbr = nc.dram_tensor("pbr", [1, DIM], BF16, kind="ExternalInput").ap()
    # per-channel columns: [128, 56]: skb 0:8, pb 8:16, b2 16:24, b1 24:56
    d_cols = nc.dram_tensor("cols", [P, 56], F32, kind="ExternalInput").ap()
    # gamma cols for STT epilogues: [128, 24]: g1 0:8, g2 8:16, g3 16:24
    d_gcol = nc.dram_tensor("gcol", [P, 24], F32, kind="ExternalInput").ap()
    # bcast lhsT rows, bf16: [2, 3*DIM]: cols i*DIM+c: row0 = -g_i, row1 = beta_i
    d_gb = nc.dram_tensor("gb", [2, 3 * DIM], BF16, kind="ExternalInput").ap()
    # collective bounce buffers: ccA = K chans 0:512 + V chans 0:512 (heads 0-7)
    ccW_in = nc.dram_tensor("ccW_in", [1, 64], BF16).ap()
    ccW_out = nc.dram_tensor("ccW_out", [2, 64], BF16).ap()
    ccA_in = nc.dram_tensor("ccA_in", [DIM, T], BF16).ap()
    ccA_out = nc.dram_tensor("ccA_out", [2 * DIM, T], BF16).ap()
    ccB_in = nc.dram_tensor("ccB_in", [DIM, T], BF16).ap()
    ccB_out = nc.dram_tensor("ccB_out", [2 * DIM, T], BF16).ap()
    d_out = nc.dram_tensor("out", [DIM, T], F32, kind="ExternalOutput").ap()
    DEBUG = bool(int(__import__("os").environ.get("KDBG", "0")))
    if DEBUG:
        d_dbg_q = nc.dram_tensor("dbg_q", [P, T], BF16, kind="ExternalOutput").ap()
        d_dbg_k = nc.dram_tensor("dbg_k", [P, T], BF16, kind="ExternalOutput").ap()
        d_dbg_e = nc.dram_tensor("dbg_e", [P, 2 * T], BF16, kind="ExternalOutput").ap()
        d_dbg_r = nc.dram_tensor("dbg_r", [1, T], BF16, kind="ExternalOutput").ap()
        d_dbg_u = nc.dram_tensor("dbg_u", [HD, T], BF16, kind="ExternalOutput").ap()
        d_dbg_o = nc.dram_tensor("dbg_o", [P, 2 * T], mybir.dt.float8e4,
                                 kind="ExternalOutput").ap()

    GROUPS = [[0, 1], [2, 3], [4, 5], [6, 7]]
    EXPW = 2 * T

    with tile.TileContext(nc, pool_alloc_mode="queue") as tc:
        with tc.tile_pool(name="glob", bufs=1) as gpool, \
             tc.tile_pool(name="tmp", bufs=2) as tpool, \
             tc.tile_pool(name="ps", bufs=3, space="PSUM") as ppool:

            # warm-up AllGather: pays the one-time ~10us CC mesh startup
            # during phase A instead of on the critical first real gather
            nc.gpsimd.collective_compute(
                "AllGather", ALU.bypass,
                replica_groups=[[0, 1], [2, 3], [4, 5], [6, 7]],
                ins=[ccW_in[:]], outs=[ccW_out[:]],
            )
            cols = gpool.tile([P, 56], F32, tag="cols", name="cols")
            gcol = gpool.tile([P, 24], F32, tag="gcol", name="gcol")
            gb = gpool.tile([2, 3 * DIM], BF16, tag="gb", name="gb")
            ones_b = gpool.tile([P, 1], BF16, tag="ones_b", name="ones_b")
            nc.vector.memset(ones_b, 1.0)
            ones_h = gpool.tile([P, 1], FP16, tag="ones_h", name="ones_h")
            nc.vector.memset(ones_h, 1.0)
            ones_row = gpool.tile([1, P], BF16, tag="ones_row", name="ones_row")
            nc.vector.memset(ones_row, 1.0)
            e8r = gpool.tile([1, P], BF16, tag="e8r", name="e8r")
            nc.vector.memset(e8r, 8.0)
            ones_t = gpool.tile([1, T], BF16, tag="ones_t", name="ones_t")
            nc.vector.memset(ones_t, 1.0)
            zrow = gpool.tile([1, P], BF16, tag="zrow", name="zrow")
            nc.vector.memset(zrow, 0.0)
            pbr = gpool.tile([1, DIM], BF16, tag="pbr", name="pbr")
            eps_t = gpool.tile([1, 1], F32, tag="eps_t", name="eps_t")
            nc.vector.memset(eps_t, EPS)
            # per-LN broadcast rhs [2, T]: row0 = mean*rstd (written), row1 = 1
            brh = [gpool.tile([2, T], BF16, tag=f"brh{i}", name=f"brh{i}")
                   for i in range(3)]
            for i in range(3):
                # whole-tile memset (1-partition access at base 1 is illegal);
                # row 0 is overwritten with mean*rstd by each LN chain
                nc.vector.memset(brh[i], 1.0)
            # long-lived activation tiles (hT aliases x1n/oT slots later)
            x1n = [gpool.tile([P, T], BF16, tag=f"x1n_{k}", name=f"x1n_{k}")
                   for k in range(8)]
            x2n = [gpool.tile([P, T], BF16, tag=f"x2n{m}", name=f"x2n{m}")
                   for m in range(8)]
            oTf = [gpool.tile([P, 2 * T], FP8, tag=f"oTf{i}", name=f"oTf{i}")
                   for i in range(4)]

            wp = [gpool.tile([P, 2 * DIM], FP8, tag=f"wp{k}", name=f"wp{k}")
                  for k in range(4)]
            x1f = [gpool.tile([P, 2 * T], FP8, tag=f"x1f{i}", name=f"x1f{i}")
                   for i in range(4)]

            C_SKB, C_PRB, C_F2B, C_F1B = 0, 8, 16, 24

            def emit_stats(stats, raw_m, sq_m, m, nk=8):
                nc.tensor.matmul(stats[0:1, :], lhsT=ones_b, rhs=raw_m,
                                 start=(m == 0), stop=(m == nk - 1))
                nc.tensor.matmul(stats[32:33, :], lhsT=ones_h, rhs=sq_m,
                                 start=(m == 0), stop=(m == nk - 1))

            def emit_ln_chain(stats, ln_i, n_feat, warm_rhs=None, warm_n=12):
                """LN stats -> (a_sb [128,T] bf16 = rstd bcast, make_b(m) fn)."""
                inv_n = 1.0 / n_feat
                if warm_rhs is not None:
                    # keep the PE HAM clock warm through the serial chain
                    hch = ppool.tile([P, T], F32, tag="mm", name="lnheat")
                    for i in range(warm_n):
                        nc.tensor.matmul(hch[0:1, :], lhsT=ones_b, rhs=warm_rhs,
                                         start=(i == 0), stop=(i == warm_n - 1))
                msq = tpool.tile([1, T], F32, tag="ln_msq", name="ln_msq", bufs=1)
                nc.scalar.activation(msq, stats[0:1, :], AF.Square, scale=inv_n)
                var = tpool.tile([1, T], F32, tag="ln_var", name="ln_var", bufs=1)
                nc.vector.scalar_tensor_tensor(var, stats[32:33, :], inv_n, msq,
                                               ALU.mult, ALU.subtract)
                lnv = tpool.tile([1, T], F32, tag="ln_lnv", name="ln_lnv", bufs=1)
                nc.scalar.activation(lnv, var, AF.Ln, bias=eps_t)
                rstd = tpool.tile([1, T], F32, tag="ln_rstd", name="ln_rstd",
                                  bufs=1)
                nc.scalar.activation(rstd, lnv, AF.Exp, scale=-0.5)
                rstd_bf = tpool.tile([1, T], BF16, tag="ln_rstdb", name="ln_rstdb",
                                     bufs=1)
                nc.vector.tensor_copy(out=rstd_bf, in_=rstd)
                # brh row0 = mean * rstd  (bf16)
                nc.vector.scalar_tensor_tensor(brh[ln_i][0:1, :], stats[0:1, :],
                                               inv_n, rstd, ALU.mult, ALU.mult)
                a_ps = ppool.tile([P, T], F32, tag="mm", name="mm")
                nc.tensor.matmul(a_ps, lhsT=ones_row, rhs=rstd_bf, start=True,
                                 stop=True)
                a_sb = tpool.tile([P, T], BF16, tag="ln_asb", name="ln_asb", bufs=1)
                nc.vector.tensor_copy(out=a_sb, in_=a_ps)

                def make_b(m):
                    b_ps = ppool.tile([P, T], F32, tag="mm", name="mm")
                    if warm_rhs is None:
                        nc.tensor.matmul(b_ps, lhsT=gb[:, ln_i * DIM + m * P:
                                                       ln_i * DIM + (m + 1) * P],
                                         rhs=brh[ln_i], start=True, stop=True)
                    else:
                        # zero-weight fills keep the clock up through the
                        # DVE-paced finals without touching the data
                        nc.tensor.matmul(b_ps, lhsT=gb[:, ln_i * DIM + m * P:
                                                       ln_i * DIM + (m + 1) * P],
                                         rhs=brh[ln_i], start=True, stop=False)
                        for fi in range(2):
                            nc.tensor.matmul(b_ps, lhsT=zrow,
                                             rhs=warm_rhs[0:1, :],
                                             start=False, stop=(fi == 1))
                    return b_ps
                return a_sb, make_b

            def emit_ln_final(raw_m, a_sb, b_ps, ln_i, m, out_tile):
                """out = g*(raw*rstd) + (-g*mr + beta), two DVE ops."""
                t1 = tpool.tile([P, T], BF16, tag="ln_t1", name="ln_t1", bufs=2)
                nc.vector.tensor_tensor(t1, raw_m, a_sb, ALU.mult)
                nc.vector.scalar_tensor_tensor(
                    out_tile, t1, gcol[:, 8 * ln_i + m:8 * ln_i + m + 1], b_ps,
                    ALU.mult, ALU.add)

            # =========== Phase A: skip-concat linear + LN1 stats ===========
            wqkvp = tc.alloc_tile_pool(name="wqkv", bufs=1)
            with tc.tile_pool(name="pha", bufs=1) as apool:
                wsk = [apool.tile([P, DIM], BF16, tag=f"wsk{k}", name=f"wsk{k}")
                       for k in range(16)]
                xs = [apool.tile([P, T], BF16, tag=f"xsh{k}", name=f"xsh{k}")
                      for k in range(16)]
                for k in range(16):
                    eng = nc.sync if k % 2 == 0 else nc.gpsimd
                    eng.dma_start(out=wsk[k], in_=d_wsk[k * P:(k + 1) * P, :])
                    eng.dma_start(out=xs[k], in_=d_xs[k * P:(k + 1) * P, :])
                    if k == 0:
                        nc.gpsimd.dma_start(out=cols, in_=d_cols)
                        nc.gpsimd.dma_start(out=gcol, in_=d_gcol)
                        nc.gpsimd.dma_start(out=gb, in_=d_gb)
                        nc.gpsimd.dma_start(out=pbr, in_=d_pbr)
                # prefetch q/k/v weights (all fp8 paired) behind phase-A tiles
                wq = [wqkvp.tile([P, 2 * DIM], FP8, tag=f"wq{k}", name=f"wq{k}")
                      for k in range(4)]
                wk = [wqkvp.tile([P, 2 * DIM], FP8, tag=f"wk{k}", name=f"wk{k}")
                      for k in range(4)]
                wv = [wqkvp.tile([P, 2 * DIM], FP8, tag=f"wv{k}", name=f"wv{k}")
                      for k in range(4)]
                for k in range(4):
                    nc.sync.dma_start(out=wk[k], in_=d_wk[k * P:(k + 1) * P, :])
                    nc.gpsimd.dma_start(out=wv[k], in_=d_wv[k * P:(k + 1) * P, :])
                    nc.sync.dma_start(out=wq[k], in_=d_wq[k * P:(k + 1) * P, :])

                raw = [apool.tile([P, T], BF16, tag=f"raw{m}", name=f"raw{m}")
                       for m in range(8)]
                sq = [apool.tile([P, T], FP16, tag=f"sq{m}", name=f"sq{m}")
                      for m in range(8)]
                # hybrid k-outer: ride the input DMA stream with all 8 psum
                # slots, then finish per-m so the drains stagger
                aslots = [ppool.tile([P, EXPW], F32, tag="big", bufs=2,
                                     name="abig") for _ in range(2)]
                aslots += [ppool.tile([P, T], F32, tag="mm", name="amm")
                           for _ in range(3)]
                aslots += [ppool.tile([P, T], F32, tag="st", name="ast", bufs=1)]

                def aslot(m):
                    if m < 4:
                        return aslots[m // 2][:, (m % 2) * T:(m % 2 + 1) * T]
                    return aslots[2 + (m - 4)]

                for k in range(12):
                    for m in range(8):
                        nc.tensor.matmul(
                            aslot(m), lhsT=wsk[k][:, m * P:(m + 1) * P],
                            rhs=xs[k], start=(k == 0), stop=False)
                stats1 = None
                for m in range(8):
                    for k in range(12, 16):
                        nc.tensor.matmul(
                            aslot(m), lhsT=wsk[k][:, m * P:(m + 1) * P],
                            rhs=xs[k], start=False, stop=(k == 15))
                    # raw = psum + skip_b (per-partition col), bf16
                    nc.vector.tensor_scalar(raw[m], aslot(m),
                                            cols[:, C_SKB + m:C_SKB + m + 1], None,
                                            ALU.add)
                    nc.scalar.activation(sq[m], raw[m], AF.Square)
                    if m == 7:
                        stats1 = ppool.tile([P, T], F32, tag="st", name="st1",
                                            bufs=1)
                    if stats1 is not None:
                        if m == 7:
                            for m2 in range(8):
                                emit_stats(stats1, raw[m2], sq[m2], m2)

                # ---- LN1 chain + finals (finals pipeline into K below) ----
                a1, make_b1 = emit_ln_chain(stats1, 0, DIM, warm_rhs=raw[0], warm_n=16)
                for m in range(8):
                    emit_ln_final(raw[m], a1, make_b1(m), 0, m, x1n[m])
                    # x16 fp8 copy of x1n pairs for QKV/proj DoubleRow
                    nc.vector.tensor_scalar(
                        x1f[m // 2][:, (m % 2) * T:(m % 2 + 1) * T],
                        x1n[m], 16.0, None, ALU.mult)

            # =========== Phase B: K, V halves + AllGathers, Q ===========
            with tc.tile_pool(name="phb", bufs=1) as bpool:
                cc_in = [ccA_in, ccB_in]
                cc_out = [ccA_out, ccB_out]
                kloc = [bpool.tile([P, T], BF16, tag=f"kl{m}", name=f"kl{m}")
                        for m in range(8)]
                vloc = [bpool.tile([P, T], BF16, tag=f"vl{i}", name=f"vl{i}")
                        for i in range(8)]
                qT = [bpool.tile([P, T], BF16, tag=f"qT{m}", name=f"qT{m}")
                      for m in range(8)]

                def emit_k(m, half):
                    # K tile m, fp8 DoubleRow over paired x1 channels
                    ps = ppool.tile([P, T], F32, tag="mm", name="mm")
                    for kp in range(4):
                        nc.tensor.matmul(
                            ps,
                            lhsT=wk[kp].rearrange("p (j c) -> p j c", j=2)
                            [:, :, m * P:(m + 1) * P],
                            rhs=x1f[kp].rearrange("p (j t) -> p j t", j=2),
                            start=(kp == 0), stop=(kp == 3),
                            perf_mode=mybir.MatmulPerfMode.DoubleRow)
                    nc.vector.tensor_scalar(kloc[m], ps, 1.0 / 512, None,
                                            ALU.mult)
                    nc.sync.dma_start(
                        out=cc_in[half][(m - 4 * half) * P:
                                        (m - 4 * half + 1) * P, :],
                        in_=kloc[m])

                def emit_v(kt, half):
                    ps = ppool.tile([P, T], F32, tag="mm", name="mm")
                    for kp in range(4):
                        nc.tensor.matmul(
                            ps,
                            lhsT=x1f[kp].rearrange("p (j t) -> p j t", j=2)
                            [:, :, kt * P:(kt + 1) * P],
                            rhs=wv[kp].rearrange("p (j c) -> p j c", j=2)
                            [:, :, half * T:(half + 1) * T],
                            start=(kp == 0), stop=(kp == 3),
                            perf_mode=mybir.MatmulPerfMode.DoubleRow)
                    nc.vector.tensor_scalar(vloc[half * 4 + kt], ps, 1.0 / 512,
                                            None, ALU.mult)
                    nc.sync.dma_start(
                        out=cc_in[half][T + kt * P:T + (kt + 1) * P, :],
                        in_=vloc[half * 4 + kt])

                # K half0 + V half0 feed AG-A as early as possible
                for m in range(4):
                    emit_k(m, 0)
                for kt in range(4):
                    emit_v(kt, 0)
                nc.gpsimd.collective_compute(
                    "AllGather", ALU.bypass, replica_groups=GROUPS,
                    ins=[cc_in[0][:]], outs=[cc_out[0][:]],
                )
                # Q for own tokens (overlaps AG-A), fp8 DoubleRow
                for m in range(8):
                    ps = ppool.tile([P, T], F32, tag="mm", name="mm")
                    for kp in range(4):
                        nc.tensor.matmul(
                            ps,
                            lhsT=wq[kp].rearrange("p (j c) -> p j c", j=2)
                            [:, :, m * P:(m + 1) * P],
                            rhs=x1f[kp].rearrange("p (j t) -> p j t", j=2),
                            start=(kp == 0), stop=(kp == 3),
                            perf_mode=mybir.MatmulPerfMode.DoubleRow)
                    nc.vector.tensor_scalar(qT[m], ps, 1.0 / 4096, None,
                                            ALU.mult)
                # K half1 + V half1 + AG-B, still ahead of attention
                for m in range(4, 8):
                    emit_k(m, 1)
                for kt in range(4):
                    emit_v(kt, 1)
                nc.gpsimd.collective_compute(
                    "AllGather", ALU.bypass, replica_groups=GROUPS,
                    ins=[cc_in[1][:]], outs=[cc_out[1][:]],
                )

                # reload gathered K/V (uniform across cores; k-token order is
                # attention-invariant). kT[m][b]: chans m*128.., token block b.
                kT = [[bpool.tile([P, T], BF16, tag=f"kT_{m}_{b}",
                                  name=f"kT_{m}_{b}")
                       for b in range(2)] for m in range(8)]
                # per-half V tiles: heads 0-7 tiles depend only on AG-A so
                # attention on heads 0-7 never waits for AG-B
                v_hf = [[bpool.tile([P, 8 * (HD + 1)], BF16, tag=f"v{half}_{kt}",
                                    name=f"v{half}_{kt}")
                         for kt in range(8)] for half in range(2)]
                for half in range(2):
                    for kt in range(8):
                        v3 = v_hf[half][kt].rearrange("p (h c) -> p h c", c=HD + 1)
                        nc.vector.memset(v3[:, :, HD:HD + 1], 1.0)

                def emit_reload(half):
                    for b in range(2):
                        for mi in range(4):
                            m = half * 4 + mi
                            nc.sync.dma_start(
                                out=kT[m][b],
                                in_=cc_out[half][b * DIM + mi * P:
                                                 b * DIM + (mi + 1) * P, :])
                        for ktl in range(4):
                            kt = b * 4 + ktl
                            v3 = v_hf[half][kt].rearrange("p (h c) -> p h c",
                                                          c=HD + 1)
                            nc.sync.dma_start(
                                out=v3[:, :, 0:HD],
                                in_=cc_out[half][b * DIM + T + ktl * P:
                                                 b * DIM + T + (ktl + 1) * P, :]
                                .rearrange("p (h c) -> p h c", c=HD))
                emit_reload(0)

                for k in range(4):
                    nc.gpsimd.dma_start(out=wp[k], in_=d_wp[k * P:(k + 1) * P, :])

                emit_reload(1)

                # =========== Phase C: attention, head-pair batched ===========
                # per pair m2 (heads 2m2, 2m2+1): 8 score slots of 2 concurrent
                # K=64 matmuls (row groups 0/64), Schraudolph exp on DVE into
                # int16-as-bf16, bf16 attnV with sums column, DVE reciprocal
                # straight off PSUM, eights-row broadcast matmul, oTf from
                # PSUM x PSUM.
                with tc.tile_pool(name="exps", bufs=8) as xpool:
                    # heat bridge while AG-A lands (PE would idle > HAM window)
                    heat = ppool.tile([P, T], F32, tag="st", name="heat", bufs=1)
                    for i in range(N_BRIDGE):
                        nc.tensor.matmul(heat, lhsT=qT[0][:, 0:P], rhs=qT[0],
                                         start=(i == 0), stop=(i == N_BRIDGE - 1))

                    pend = None   # (m2, poA, poB, rhbA, rhbB) awaiting bc/oTf

                    def emit_scores(m2):
                        es = []
                        for j in range(8):
                            th, col = j // 4, (j % 4) * P
                            big = ppool.tile([P, EXPW], F32, tag="big", bufs=2,
                                             name="sc")
                            nc.tensor.matmul(
                                big[:, 0:T],
                                lhsT=kT[m2][th][0:HD, col:col + P],
                                rhs=qT[m2][0:HD, :], start=True, stop=True)
                            nc.tensor.matmul(
                                big[:, T:2 * T],
                                lhsT=kT[m2][th][HD:P, col:col + P],
                                rhs=qT[m2][HD:P, :], start=True, stop=True)
                            e = xpool.tile([P, EXPW], BF16, tag="exp", name="exp")
                            if j % 2 == 0 or j == 7:
                                # Schraudolph bit-exp on DVE
                                nc.vector.tensor_scalar(e.bitcast(I16), big,
                                                        EXP_A, EXP_B,
                                                        ALU.mult, ALU.add)
                            else:
                                # table exp on ACT: same ~1us/tile, other engine
                                nc.scalar.activation(e, big, AF.Exp)
                            es.append(e)
                        return es

                    def emit_attnv(m2, es):
                        half = m2 // 4
                        pos = []
                        for hh2 in range(2):
                            hh = (m2 % 4) * 2 + hh2
                            po = ppool.tile([P, T], F32, tag="mm", name="mm")
                            for j in range(8):
                                nc.tensor.matmul(
                                    po[0:HD + 1, :],
                                    lhsT=v_hf[half][j]
                                    [:, hh * (HD + 1):(hh + 1) * (HD + 1)],
                                    rhs=es[j][:, hh2 * T:(hh2 + 1) * T],
                                    start=(j == 0), stop=(j == 7))
                            # den must reach a base-0 tile before the custom
                            # recip op (it mishandles base_partition != 0);
                            # stage it + the o_unnorm drain on ACT
                            den = tpool.tile([1, T], F32, tag=f"den{hh2}",
                                             name="den", bufs=2)
                            nc.scalar.activation(den, po[HD:HD + 1, :], AF.Copy)
                            rh = tpool.tile([1, T], F32, tag=f"rh{hh2}",
                                            name="rh", bufs=2)
                            nc.vector.reciprocal_approx_fast(out=rh, in_=den)
                            rhb = tpool.tile([1, T], BF16, tag=f"rhb{hh2}",
                                             name="rhb", bufs=2)
                            nc.scalar.activation(rhb, rh, AF.Copy)
                            oU = tpool.tile([HD, T], BF16, tag="oub",
                                            name="oub", bufs=4)
                            nc.scalar.activation(oU, po[0:HD, :], AF.Copy)
                            pos.append((oU, rhb))
                        return (m2, pos[0][0], pos[1][0], pos[0][1], pos[1][1])

                    def finish_pair(m2, oUA, oUB, rhA, rhB):
                        # bc = 8/den broadcast; oTf = (o_unnorm/den)*8 in fp8
                        bcA = ppool.tile([P, T], F32, tag="st", name="bcA",
                                         bufs=1)
                        nc.tensor.matmul(bcA[0:HD, :], lhsT=e8r[:, 0:HD],
                                         rhs=rhA, start=True, stop=True)
                        bcB = ppool.tile([P, T], F32, tag="mm", name="bcB")
                        nc.tensor.matmul(bcB[0:HD, :], lhsT=e8r[:, 0:HD],
                                         rhs=rhB, start=True, stop=True)
                        sl = slice((m2 % 2) * T, (m2 % 2 + 1) * T)
                        nc.vector.tensor_tensor(
                            oTf[m2 // 2][0:HD, sl], oUA, bcA[0:HD, :], ALU.mult)
                        nc.vector.tensor_tensor(
                            oTf[m2 // 2][HD:P, sl], oUB, bcB[0:HD, :], ALU.mult)

                    for m2 in range(8):
                        es = emit_scores(m2)
                        if DEBUG and m2 == 0:
                            nc.gpsimd.dma_start(out=d_dbg_q, in_=qT[0])
                            nc.gpsimd.dma_start(out=d_dbg_k, in_=kT[0][0])
                            nc.gpsimd.dma_start(out=d_dbg_e, in_=es[0])
                        pend2 = emit_attnv(m2, es)
                        if DEBUG and m2 == 0:
                            nc.gpsimd.dma_start(out=d_dbg_u, in_=pend2[1])
                        # previous pair's bc/oTf AFTER this pair's attnV: the
                        # den->recip chain latency hides under real PE work
                        if pend is not None:
                            finish_pair(*pend)
                        pend = pend2
                    finish_pair(*pend)
                    if DEBUG:
                        nc.gpsimd.dma_start(out=d_dbg_o, in_=oTf[0])

                    # proj k-outer over 5 psum slots starts while the last
                    # head's epilogue drains
                    pj = [ppool.tile([P, EXPW], F32, tag="big", bufs=2,
                                     name="pjbig") for _ in range(2)]
                    pj += [ppool.tile([P, T], F32, tag="st", name="pjst", bufs=1)]

                    def pjslot(m):
                        if m < 4:
                            return pj[m // 2][:, (m % 2) * T:(m % 2 + 1) * T]
                        return pj[2]

                    for kp in range(4):
                        for m in range(5):
                            nc.tensor.matmul(
                                pjslot(m),
                                lhsT=wp[kp].rearrange("p (j c) -> p j c", j=2)
                                [:, :, m * P:(m + 1) * P],
                                rhs=oTf[kp].rearrange("p (j t) -> p j t", j=2),
                                start=(kp == 0), stop=False,
                                perf_mode=mybir.MatmulPerfMode.DoubleRow)
                    for m in range(5):
                        # + 256*proj_b via bf16 K=1 row (closes the group)
                        nc.tensor.matmul(pjslot(m), lhsT=pbr[:, m * P:(m + 1) * P],
                                         rhs=ones_t, start=False, stop=True)
            wqkvp.release()

            # =========== Phase D: proj m5..7 + residual + LN2 ===========
            w1pool = tc.alloc_tile_pool(name="w1p", bufs=1)
            w1 = [w1pool.tile([P, HIDDEN], BF16, tag=f"w1{k}", name=f"w1{k}")
                  for k in range(8)]
            for k in range(8):
                eng = nc.sync if k % 2 == 0 else nc.gpsimd
                eng.dma_start(out=w1[k], in_=d_w1[k * P:(k + 1) * P, :])
            with tc.tile_pool(name="phd", bufs=1) as dpool:
                x2r = [dpool.tile([P, T], BF16, tag=f"x2r{m}", name=f"x2r{m}")
                       for m in range(8)]
                x2sq = [dpool.tile([P, T], FP16, tag=f"x2sq{m}", name=f"x2sq{m}")
                        for m in range(8)]
                pj5 = [None] * 3
                for mi in range(3):
                    m = 5 + mi
                    ps = ppool.tile([P, T], F32, tag="mm", name="mm")
                    for kp in range(4):
                        nc.tensor.matmul(
                            ps,
                            lhsT=wp[kp].rearrange("p (j c) -> p j c", j=2)
                            [:, :, m * P:(m + 1) * P],
                            rhs=oTf[kp].rearrange("p (j t) -> p j t", j=2),
                            start=(kp == 0), stop=False,
                            perf_mode=mybir.MatmulPerfMode.DoubleRow)
                    nc.tensor.matmul(ps, lhsT=pbr[:, m * P:(m + 1) * P],
                                     rhs=ones_t, start=False, stop=True)
                    pj5[mi] = ps
                stats2 = None
                for m in range(8):
                    ps = pjslot(m) if m < 5 else pj5[m - 5]
                    # x2r = psum/256 + x1n (psum carries 256*proj_b)
                    nc.vector.scalar_tensor_tensor(
                        x2r[m], ps, 1.0 / 256, x1n[m], ALU.mult, ALU.add)
                    nc.scalar.activation(x2sq[m], x2r[m], AF.Square)
                    if m == 0:
                        stats2 = ppool.tile([P, T], F32, tag="st", name="st2",
                                            bufs=1)
                    emit_stats(stats2, x2r[m], x2sq[m], m)

                a2, make_b2 = emit_ln_chain(stats2, 1, DIM, warm_rhs=x2r[0])
                for m in range(8):
                    emit_ln_final(x2r[m], a2, make_b2(m), 1, m, x2n[m])

            # =========== Phase E: MLP + LN3 ===========
            with tc.tile_pool(name="phe", bufs=1) as epool:
                hT = [epool.tile([P, T], BF16, tag=f"hT{i}", name=f"hT{i}")
                      for i in range(32)]
                pf = None
                for mm in range(32):
                    if mm < 4:
                        # k-outer for the first 4 out-tiles (pipelines with LN2)
                        if mm == 0:
                            pf = [ppool.tile([P, EXPW], F32, tag="big", bufs=2,
                                             name="fbig") for _ in range(2)]
                            for k in range(8):
                                for j in range(4):
                                    nc.tensor.matmul(
                                        pf[j // 2][:, (j % 2) * T:(j % 2 + 1) * T],
                                        lhsT=w1[k][:, j * P:(j + 1) * P],
                                        rhs=x2n[k], start=(k == 0), stop=(k == 7))
                        ps = pf[mm // 2][:, (mm % 2) * T:(mm % 2 + 1) * T]
                    else:
                        ps = ppool.tile([P, T], F32, tag="mm", name="mm")
                        for k in range(8):
                            nc.tensor.matmul(ps, lhsT=w1[k][:, mm * P:(mm + 1) * P],
                                             rhs=x2n[k], start=(k == 0),
                                             stop=(k == 7))
                    nc.scalar.activation(hT[mm], ps, AF.Gelu,
                                         bias=cols[:, C_F1B + mm:C_F1B + mm + 1])

                # ---- fc2: two k-outer passes of 4 out-tiles, w2 streamed
                # through 4 rotating SBUF tiles (re-read on pass 2); pass-1
                # drains overlap pass-2 matmuls ----
                x3sq = [epool.tile([P, T], FP16, tag=f"x3sq{m}", name=f"x3sq{m}")
                        for m in range(8)]
                x3r = [epool.tile([P, T], BF16, tag=f"x3r{m}", name=f"x3r{m}")
                       for m in range(8)]
                f2ps = {}
                for p2 in range(2):
                    if p2 == 0:
                        slots = [ppool.tile([P, EXPW], F32, tag="big", bufs=2,
                                            name="f2big") for _ in range(2)]
                        sl = lambda m: slots[m // 2][:, (m % 2) * T:
                                                     (m % 2 + 1) * T]
                    else:
                        slots = [ppool.tile([P, T], F32, tag="mm", name="f2mm")
                                 for _ in range(3)]
                        slots += [ppool.tile([P, T], F32, tag="st", name="f2st",
                                             bufs=1)]
                        sl = lambda m: slots[m - 4]
                    for k in range(32):
                        w2t = epool.tile([P, T], BF16, tag=f"w2r{k % 4}",
                                         name=f"w2_{p2}_{k}")
                        eng = nc.sync if k % 2 == 0 else nc.gpsimd
                        eng.dma_start(out=w2t,
                                      in_=d_w2[k * P:(k + 1) * P,
                                               p2 * T:(p2 + 1) * T])
                        for mi in range(4):
                            m = p2 * 4 + mi
                            nc.tensor.matmul(sl(m),
                                             lhsT=w2t[:, mi * P:(mi + 1) * P],
                                             rhs=hT[k], start=(k == 0),
                                             stop=(k == 31))
                    if p2 == 0:
                        # stats live in a freed pass-1 "big" bank so the
                        # stats matmuls interleave with pass-2 matmuls
                        stats3 = ppool.tile([P, EXPW], F32, tag="big", bufs=2,
                                            name="st3")[:, 0:T]
                    for mi in range(4):
                        m = p2 * 4 + mi
                        f2ps[m] = sl(m)
                        nc.vector.scalar_tensor_tensor(
                            x3r[m], f2ps[m], cols[:, C_F2B + m:C_F2B + m + 1],
                            x2n[m], ALU.add, ALU.add)
                        nc.scalar.activation(x3sq[m], x3r[m], AF.Square)
                        emit_stats(stats3, x3r[m], x3sq[m], m)

                a3, make_b3 = emit_ln_chain(stats3, 2, DIM, warm_rhs=x3r[0], warm_n=24)
                for m in range(8):
                    xo = tpool.tile([P, T], F32, tag="xo", name="xo", bufs=2)
                    emit_ln_final(x3r[m], a3, make_b3(m), 2, m, xo)
                    eng = nc.sync if m % 2 == 0 else nc.gpsimd
                    eng.dma_start(out=d_out[m * P:(m + 1) * P, :], in_=xo)
            w1pool.release()

    # Steer the act-table selector: keep dict ORDER (act_func_set_id is the
    # positional index into act_info.json) but hide Exp/Ln from the small
    # tables so both resolve to the combined natural_log_exp_and_others set
    # and the attention/LN loop stops thrashing table loads.
    import concourse.hw_specs as hw_specs
    tabs = dict(hw_specs.get_activation_tables("gen3"))
    steered = {}
    for name, fns in tabs.items():
        fns = set(fns)
        if name != "natural_log_exp_and_others":
            fns.discard(AF.Exp)
            fns.discard(AF.Ln)
        steered[name] = fns
    import functools
    _orig = hw_specs.get_activation_tables
    patched = functools.lru_cache(None)(
        lambda arch: steered if arch == "gen3" else _orig(arch))
    hw_specs.get_activation_tables = patched
    import concourse.bacc as bacc_mod
    bacc_mod.get_activation_tables = patched

    nc.compile()
    _BUILT = nc
    return nc


def _pack_col(vec, ncols):
    """[N] per-channel vector -> [128, N//128] tile layout (channel c -> [c%128, c//128])."""
    return np.ascontiguousarray(vec.reshape(ncols, P).T.astype(np.float32))


def _prep_in_maps(inputs):
    bf = ml_dtypes.bfloat16
    x = np.asarray(inputs["x"], np.float32)
    skip = np.asarray(inputs["skip"], np.float32)
    xs = np.concatenate([x, skip], axis=2)          # [4, 1024, 2048]

    wsk = np.asarray(inputs["skip_w"], np.float32).astype(bf)
    qkv = np.asarray(inputs["qkv_w"], np.float32)
    f8 = ml_dtypes.float8_e4m3

    def pack8(w, s):
        return np.ascontiguousarray(
            (w * s).reshape(4, 2, P, DIM)
            .transpose(0, 2, 1, 3).reshape(4 * P, 2 * DIM)).astype(f8)

    wq = pack8(qkv[:, :DIM] * SCALE, 256.0)
    wk = pack8(qkv[:, DIM:2 * DIM], 32.0)
    wv = pack8(qkv[:, 2 * DIM:], 32.0)
    wp = np.ascontiguousarray(
        (np.asarray(inputs["proj_w"], np.float32) * 32.0).reshape(4, 2, P, DIM)
        .transpose(0, 2, 1, 3).reshape(4 * P, 2 * DIM)).astype(f8)
    w1 = np.asarray(inputs["fc1_w"], np.float32).astype(bf)
    w2 = np.asarray(inputs["fc2_w"], np.float32).astype(bf)
    pbrow = (np.asarray(inputs["proj_b"], np.float32) * 256.0)[None, :].astype(bf)

    cols = np.zeros((P, 56), np.float32)
    cols[:, 0:8] = _pack_col(np.asarray(inputs["skip_b"], np.float32), 8)
    cols[:, 8:16] = _pack_col(np.asarray(inputs["proj_b"], np.float32), 8)
    cols[:, 16:24] = _pack_col(np.asarray(inputs["fc2_b"], np.float32), 8)
    cols[:, 24:56] = _pack_col(np.asarray(inputs["fc1_b"], np.float32), 32)

    gcolv = np.zeros((P, 24), np.float32)
    gcolv[:, 0:8] = _pack_col(np.asarray(inputs["ln1_g"], np.float32), 8)
    gcolv[:, 8:16] = _pack_col(np.asarray(inputs["ln2_g"], np.float32), 8)
    gcolv[:, 16:24] = _pack_col(np.asarray(inputs["ln3_g"], np.float32), 8)

    gbv = np.zeros((2, 3 * DIM), np.float32)
    for i, (gk, bk) in enumerate([("ln1_g", "ln1_b"), ("ln2_g", "ln2_b"),
                                  ("ln3_g", "ln3_b")]):
        gbv[0, i * DIM:(i + 1) * DIM] = -np.asarray(inputs[gk], np.float32)
        gbv[1, i * DIM:(i + 1) * DIM] = np.asarray(inputs[bk], np.float32)

    in_maps = []
    for c in range(NC):
        b, h = c // 2, c % 2
        seq = xs[b][h * T:(h + 1) * T]               # own 512 tokens
        xsT = np.ascontiguousarray(seq.T).astype(bf)  # [2048, 512]
        in_maps.append({
            "xs": xsT, "wsk": wsk, "wq": wq, "wk": wk, "wv": wv,
            "wp": wp, "w1": w1, "w2": w2, "pbr": pbrow, "cols": cols,
            "gcol": gcolv, "gb": gbv.astype(bf),
        })
    return in_maps


def run(inputs, trace=False, trace_kwargs=None):
    nc = build()
    in_maps = _prep_in_maps(inputs)
    res = run_bass_kernel_spmd(nc, in_maps, core_ids=list(range(NC)),
                               trace=trace, **(trace_kwargs or {}))
    full = np.empty((B, L, DIM), np.float32)
    for c in range(NC):
        b, h = c // 2, c % 2
        full[b, h * T:(h + 1) * T, :] = res.results[c]["out"].T
    return full, res


def kernel(**inputs):
    out, _ = run(inputs, trace=False)
    return out

